# revision 14
# baseline (speedup 1.0000x reference)
"""MoE (dense routing) Trainium2 kernel: 8-core data-parallel over tokens.

Problem: nn_MixtureOfExperts_33011118637071
  N=16384 tokens, D=256 model dim, E=8 experts, H=128 gate hidden.
  gate   = softmax(relu(x @ Wg1 + bg1) @ Wg2 + bg2)          [N, E]
  h_e    = relu(x @ W1[e] + b1[e])                           [N, D]
  y      = sum_e gate[:, e] * (h_e @ W2[e] + b2[e])          [N, D]

Strategy (per core, 2048 tokens):
  Feature-major layout (features on partitions, tokens on the free dim) so
  the two expert GEMMs chain without transposes; x is transposed on the
  host as part of sharding and the output transposed back on gather.
  Matmuls run in float32r (full PE rate, ~tf32 accuracy).

  v2 over the original baseline:
  - All K<=8 one-hot/broadcast matmuls (gate-row broadcast, b2 init,
    exp-sum, 1/sum broadcast) are packed 4-per-pass into disjoint 32-row
    strips of the PE array via tile_position so they overlap instead of
    each paying a full N=512 streaming pass. The gate logits are computed
    replicated 4x across partition strips (Wg2 replicated host-side) so
    every strip has the gate rows it needs.
  - The softmax normalization is applied once per token tile to the
    replicated exp rows (one DVE multiply against the PE-broadcast 1/sum),
    so expert outputs accumulate already-normalized in PSUM and the final
    evacuation is a ScalarE copy instead of a VectorE multiply - removing
    the end-of-tile DVE dependency that stalled the next tile's PSUM bank
    reuse.
  - exp-sum matmuls interleave into the gate phase; 1/sum broadcasts are
    software-pipelined one tile ahead so no PE stall waits on them.
  - All small constants (gate weights, one-hot masks, b1, b2 blocks) are
    fused into one [128, 1043] host-built tensor loaded with a single DMA.
  - PSUM: quad pool (4 banks) for gate/broadcast passes + 2 output-accum
    banks + 2 hidden-layer banks = 8.
"""
import numpy as np

import bass_rust
import concourse.bass as bass
import concourse.mybir as mybir
import concourse.tile as tile
from concourse.bass_utils import run_bass_kernel_spmd

F32 = mybir.dt.float32
F32R = mybir.dt.float32r
AF = mybir.ActivationFunctionType

N, D, E, H = 16384, 256, 8, 128
NCORES = 8
TPC = N // NCORES          # tokens per core
T = 512                    # token tile (max fp32 moving free dim)
NT = TPC // T              # token tiles per core
KC = D // 128              # 128-row chunks of the model dim

# fused gate/const tensor column layout ([128, GC_W] fp32)
GC_WG1 = 0                 # 256 cols: Wg1 as [p, kc*H + h]
GC_WG2 = 256               # 128 cols: Wg2 replicated into strips 32s+(0..7)
GC_BG1 = 384               # 1 col: bg1
GC_BG2 = 385               # 1 col: bg2 replicated into strips
GC_OH1 = 386               # 3x128 cols: one-hot rows; pass p strip j ->
                           # expert 3p+j (pass 2 only strips 0/1)
GC_OND = 770               # 128 cols: sum-selector; strip s rows have ones
                           # in column 32s only, so the exp-sum matmul's
                           # output lands at partition 32s with col group 0
GC_ON1 = 898               # 128 cols: ones rows at partitions {0,32,64,96}
GC_B2 = 1026               # 256 cols: b2 128-col blocks at strips 0 (mc=0), 1 (mc=1)
GC_B1 = 1282               # 16 cols: b1 as [p, e*KC + kc]
GC_W = 1298

# broadcast pass structure: expert e -> (pass e//3, strip e%3) except
# experts 6,7 -> (pass 2, strips 0,1)
def _pass_strip(e):
    return (e // 3, e % 3) if e < 6 else (2, e - 6)

_CTR = [0]


def _split_multi_waits(nc, max_waits=1):
    """This container's walrus rejects >1 sync-wait per instruction; hoist
    extras onto fresh same-engine NoOps placed just before the waiter."""
    for fn in nc.m.functions:
        for bb in fn.blocks:
            out = []
            for inst in bb.instructions:
                si = inst.sync_info
                waits = list(si.on_wait) if si is not None and si.on_wait else []
                if len(waits) > max_waits:
                    for w in waits[:-max_waits]:
                        _CTR[0] += 1
                        nop = bass_rust.InstNoOp(
                            name=f"I-waitfix-{_CTR[0]}", ins=[], outs=[])
                        nop.engine = inst.engine
                        nop.sync_info = mybir.SyncInfo(on_wait=[w], on_update=[])
                        nc.register_instruction(nop)
                        out.append(nop)
                    si.on_wait = waits[-max_waits:]
                out.append(inst)
            bb.instructions = out


def build_nc(repeat: int = 1):
    nc = bass.Bass("TRN2", target_bir_lowering=False, debug=False,
                   num_devices=NCORES)

    xT_d = nc.dram_tensor("xT", [D, TPC], F32, kind="ExternalInput")
    gc_d = nc.dram_tensor("gc", [128, GC_W], F32, kind="ExternalInput")
    W1_d = nc.dram_tensor("W1", [E, D, D], F32, kind="ExternalInput")
    W2_d = nc.dram_tensor("W2", [E, D, D], F32, kind="ExternalInput")
    yT_d = nc.dram_tensor("yT", [D, TPC], F32, kind="ExternalOutput")

    with tile.TileContext(nc) as tc:
        with (
            nc.allow_low_precision(reason="float32r matmul operands"),
            tc.tile_pool(name="wpool", bufs=1) as wp,
            tc.tile_pool(name="work", bufs=3) as sb,
            tc.tile_pool(name="gbuf", bufs=NT + 1) as gb,
            tc.tile_pool(name="hbuf", bufs=4) as hb,
            tc.tile_pool(name="obuf", bufs=4) as ob,
            tc.tile_pool(name="xpool", bufs=2) as xp,
            tc.tile_pool(name="quad", bufs=3, space="PSUM") as quad,
            tc.tile_pool(name="phid", bufs=3, space="PSUM") as phid,
            tc.tile_pool(name="pout", bufs=2, space="PSUM") as pout,
        ):
            gcx = wp.tile([128, GC_W], F32R, tag="gc")
            nc.sync.dma_start(gcx[:, :], gc_d[:, :].bitcast(F32R))

            def wg1_ap(kc):
                return gcx[:, GC_WG1 + kc * H:GC_WG1 + (kc + 1) * H]
            wg2r = gcx[:, GC_WG2:GC_WG2 + 128]
            bg1 = gcx[:, GC_BG1:GC_BG1 + 1].bitcast(F32)
            bg2r = gcx[:, GC_BG2:GC_BG2 + 1].bitcast(F32)

            def oh_ap(e):
                p, j = _pass_strip(e)
                base = GC_OH1 + 128 * p
                return gcx[32 * j:32 * j + 8, base:base + 128]

            def ond_ap(ti):
                return gcx[32 * ti:32 * ti + 8, GC_OND:GC_OND + 32 * ti + 1]

            def on1_ap(ti):
                return gcx[32 * ti:32 * ti + 1, GC_ON1:GC_ON1 + 128]

            def b2blk(mc):
                return gcx[32 * mc:32 * mc + 8,
                           GC_B2 + 128 * mc:GC_B2 + 128 * (mc + 1)]

            def b1bias(e, mc):
                c = GC_B1 + e * KC + mc
                return gcx[:, c:c + 1].bitcast(F32)

            w1 = wp.tile([128, E, KC, D], F32R, tag="w1")
            w2 = wp.tile([128, E, KC, D], F32R, tag="w2")

            def load_expert_weights(first_only=False, skip_first=False):
                w1src = W1_d.ap().rearrange(
                    "e (kc p) d -> p e kc d", p=128).bitcast(F32R)
                w2src = W2_d.ap().rearrange(
                    "e (kc p) d -> p e kc d", p=128).bitcast(F32R)
                for e in range(1 if skip_first else 0,
                               1 if first_only else E):
                    nc.sync.dma_start(w1[:, e, :, :], w1src[:, e, :, :])
                    nc.sync.dma_start(w2[:, e, :, :], w2src[:, e, :, :])

            def gate(xt, ti, rep, invs):
                """Gate logits/exp for tile ti, gate rows replicated x4
                across strips; exp-sum matmul on strip ti + reciprocal."""
                tok = slice(ti * T, (ti + 1) * T)
                pg1 = quad.tile([128, T], F32, tag="q", name=f"pg1_{rep}_{ti}")
                for kc in range(KC):
                    nc.tensor.matmul(pg1[:, :], wg1_ap(kc), xt[:, kc, tok],
                                     start=(kc == 0), stop=(kc == KC - 1))
                rh = sb.tile([H, T], F32R, tag="rh", name=f"rh_{rep}_{ti}")
                nc.scalar.activation(rh[:, :], pg1[:, :], AF.Relu, bias=bg1)
                pg2 = quad.tile([128, T], F32, tag="q", name=f"pg2_{rep}_{ti}")
                nc.tensor.matmul(pg2[:, :], wg2r, rh[:, :],
                                 start=True, stop=True)
                expl = gb.tile([128, T], F32R, tag="expl",
                               name=f"expl_{rep}_{ti}")
                nc.scalar.activation(expl[:, :], pg2[:, :], AF.Exp, bias=bg2r)
                qs = quad.tile([128, T], F32, tag="q", name=f"qs_{rep}_{ti}")
                nc.tensor.matmul(qs[0:32 * ti + 1, :], ond_ap(ti),
                                 expl[32 * ti:32 * ti + 8, :],
                                 start=True, stop=True,
                                 tile_position=(32 * ti, 0))
                nc.vector.reciprocal(invs[32 * ti:32 * ti + 1, :],
                                     qs[32 * ti:32 * ti + 1, :])
                return expl

            def normalize(ti, rep, invs, expl):
                """Broadcast 1/sum to 128 partitions (K=1 matmul on strip ti)
                and scale the replicated exp rows in place: expl becomes the
                normalized gate, so downstream accumulation needs no final
                renormalization."""
                pv = quad.tile([128, T], F32, tag="q", name=f"pv_{rep}_{ti}")
                nc.tensor.matmul(pv[:, :], on1_ap(ti),
                                 invs[32 * ti:32 * ti + 1, :],
                                 start=True, stop=True,
                                 tile_position=(32 * ti, 0))
                nc.vector.tensor_mul(expl[:, :], expl[:, :], pv[:, :])

            def experts_compute(xt, ti, rep, expl, post_e1=None):
                tok = slice(ti * T, (ti + 1) * T)

                def bcast_pass(base):
                    out = []
                    for e in range(base, min(base + 3, E)):
                        _, j = _pass_strip(e)
                        pt = quad.tile([128, T], F32, tag="q",
                                       name=f"pgb_{rep}_{ti}_{e}")
                        nc.tensor.matmul(pt[:, :], oh_ap(e),
                                         expl[32 * j:32 * j + 8, :],
                                         start=True, stop=True,
                                         tile_position=(32 * j, 0))
                        out.append(pt)
                    return out

                pgb = bcast_pass(0)
                py = None
                for e in range(E):
                    if e in (3, 6):
                        pgb = bcast_pass(e)
                    if e == 2 and post_e1 is not None:
                        post_e1()
                    pt = pgb[e % 3 if e < 6 else e - 6]
                    hs = hb.tile([128, KC, T], F32R, tag="hs",
                                 name=f"hs_{rep}_{ti}_{e}")
                    for mc in range(KC):
                        ph = phid.tile([128, T], F32, tag="ph",
                                       name=f"ph_{rep}_{ti}_{e}_{mc}")
                        for kc in range(KC):
                            nc.tensor.matmul(
                                ph[:, :], w1[:, e, kc, mc * 128:(mc + 1) * 128],
                                xt[:, kc, tok],
                                start=(kc == 0), stop=(kc == KC - 1))
                        nc.scalar.activation(hs[:, mc, :], ph[:, :], AF.Relu,
                                             bias=b1bias(e, mc))
                    # one gate multiply per expert: the [128,T] broadcast
                    # tile replays (stride 0) across the mc dimension
                    ptb = pt[:, :].rearrange(
                        "p (a t) -> p a t", a=1).to_broadcast([128, KC, T])
                    nc.vector.tensor_mul(hs[:, :, :], hs[:, :, :], ptb)
                    if e == 0:
                        # b2 init after e0's first-layer matmuls: gives the
                        # previous tile's output copies time to free the
                        # banks before the accumulation group opens.
                        py = [pout.tile([128, T], F32, tag="py",
                                        name=f"py{mc}_{rep}_{ti}")
                              for mc in range(KC)]
                        for mc in range(KC):
                            nc.tensor.matmul(py[mc][:, :], b2blk(mc),
                                             expl[32 * mc:32 * mc + 8, :],
                                             start=True, stop=False,
                                             tile_position=(32 * mc, 0))
                    for mc in range(KC):
                        for kc in range(KC):
                            nc.tensor.matmul(
                                py[mc][:, :],
                                w2[:, e, kc, mc * 128:(mc + 1) * 128],
                                hs[:, kc, :],
                                start=False,
                                stop=(e == E - 1 and kc == KC - 1))
                return py

            def finalize(ti, rep, py):
                tok = slice(ti * T, (ti + 1) * T)
                for mc in range(KC):
                    ot = ob.tile([128, T], F32, tag="ot",
                                 name=f"ot_{rep}_{ti}_{mc}")
                    # split the two evacuations across ScalarE/VectorE so
                    # neither engine serializes the tile tail
                    if mc == 0:
                        nc.scalar.activation(ot[:, :], py[mc][:, :], AF.Copy)
                    else:
                        nc.vector.tensor_copy(ot[:, :], py[mc][:, :])
                    nc.gpsimd.dma_start(yT_d[mc * 128:(mc + 1) * 128, tok],
                                        ot[:, :])

            for rep in range(repeat):
                xt = xp.tile([128, KC, TPC], F32R, tag="xt", name=f"xt{rep}")
                xsrc = xT_d.ap().rearrange(
                    "(kc p) t -> p kc t", p=128).bitcast(F32R)
                nc.sync.dma_start(xt[:, :, 0:T], xsrc[:, :, 0:T])
                if rep == 0:
                    load_expert_weights(first_only=True)
                for ti in range(1, NT):
                    tok = slice(ti * T, (ti + 1) * T)
                    nc.sync.dma_start(xt[:, :, tok], xsrc[:, :, tok])
                invs = sb.tile([128, T], F32R, tag="invs", name=f"invs_{rep}")
                expls = []
                for ti in range(NT):
                    expls.append(gate(xt, ti, rep, invs))
                    # normalize lags one tile behind the gate so the
                    # reciprocal is ready and the PE never waits on it
                    if ti >= 1:
                        normalize(ti - 1, rep, invs, expls[ti - 1])
                if rep == 0:
                    load_expert_weights(skip_first=True)
                for ti in range(NT):
                    # the last tile's normalize is deferred into tile 0's
                    # expert phase so its reciprocal has time to finish and
                    # its PSUM bank never gates the broadcast-pass ring
                    post = ((lambda: normalize(NT - 1, rep, invs,
                                               expls[NT - 1]))
                            if ti == 0 else None)
                    py = experts_compute(xt, ti, rep, expls[ti],
                                         post_e1=post)
                    finalize(ti, rep, py)

    _split_multi_waits(nc)
    return nc


_NC_CACHE = None


def _get_nc():
    global _NC_CACHE
    if _NC_CACHE is None:
        _NC_CACHE = build_nc()
    return _NC_CACHE


def make_in_maps(x, Wg1, bg1, Wg2, bg2, W1, b1, W2, b2):
    x = np.ascontiguousarray(np.asarray(x, dtype=np.float32))
    xT = np.ascontiguousarray(x.T)           # [D, N]
    Wg1 = np.asarray(Wg1, np.float32)
    bg1 = np.asarray(bg1, np.float32)
    Wg2 = np.asarray(Wg2, np.float32)
    bg2 = np.asarray(bg2, np.float32)
    b1 = np.asarray(b1, np.float32)
    b2 = np.asarray(b2, np.float32)

    gc = np.zeros((128, GC_W), np.float32)
    # Wg1 [D, H] -> [p, kc*H + h]
    gc[:, GC_WG1:GC_WG1 + KC * H] = (
        Wg1.reshape(KC, 128, H).transpose(1, 0, 2).reshape(128, KC * H))
    # Wg2 replicated: wg2r[h, 32s+k] = Wg2[h, k]; bg2 likewise per strip
    for s in range(4):
        gc[:, GC_WG2 + 32 * s:GC_WG2 + 32 * s + 8] = Wg2
        gc[32 * s:32 * s + 8, GC_BG2] = bg2
    gc[:, GC_BG1] = bg1
    # one-hot strips: expert e lives in pass e//3 at strip e%3 (6,7: pass 2
    # strips 0/1); within its [8,128] strip block, row e is ones
    for e in range(E):
        p, j = _pass_strip(e)
        gc[32 * j + e, GC_OH1 + 128 * p:GC_OH1 + 128 * (p + 1)] = 1.0
    for j in range(4):
        gc[32 * j:32 * j + 8, GC_OND + 32 * j] = 1.0
        gc[32 * j, GC_ON1:GC_ON1 + 128] = 1.0
    # b2 blocks: strip mc holds b2[:, mc*128:(mc+1)*128]
    for mc in range(KC):
        gc[32 * mc:32 * mc + 8,
           GC_B2 + 128 * mc:GC_B2 + 128 * (mc + 1)] = b2[:, mc * 128:(mc + 1) * 128]
    # b1 as [p, e*KC + kc]
    gc[:, GC_B1:GC_B1 + E * KC] = (
        b1.reshape(E, KC, 128).transpose(2, 0, 1).reshape(128, E * KC))

    shared = {
        "gc": np.ascontiguousarray(gc),
        "W1": np.ascontiguousarray(np.asarray(W1, np.float32)),
        "W2": np.ascontiguousarray(np.asarray(W2, np.float32)),
    }
    return [
        {"xT": np.ascontiguousarray(xT[:, c * TPC:(c + 1) * TPC]), **shared}
        for c in range(NCORES)
    ]


def gather_output(results):
    out = np.empty((N, D), np.float32)
    for c in range(NCORES):
        out[c * TPC:(c + 1) * TPC, :] = results[c]["yT"].T
    return out


def kernel(x, Wg1, bg1, Wg2, bg2, W1, b1, W2, b2):
    nc = _get_nc()
    in_maps = make_in_maps(x, Wg1, bg1, Wg2, bg2, W1, b1, W2, b2)
    r = run_bass_kernel_spmd(nc, in_maps, list(range(NCORES)))
    return gather_output(r.results)


# revision 17
# speedup vs baseline: 1.2229x; 1.2229x over previous
"""MoE (dense routing) Trainium2 kernel: 8-core data-parallel over tokens.

Problem: nn_MixtureOfExperts_33011118637071
  N=16384 tokens, D=256 model dim, E=8 experts, H=128 gate hidden.
  gate   = softmax(relu(x @ Wg1 + bg1) @ Wg2 + bg2)          [N, E]
  h_e    = relu(x @ W1[e] + b1[e])                           [N, D]
  y      = sum_e gate[:, e] * (h_e @ W2[e] + b2[e])          [N, D]

Strategy (per core, 2048 tokens):
  Feature-major layout (features on partitions, tokens on the free dim) so
  the two expert GEMMs chain without transposes; x is transposed on the
  host as part of sharding and the output transposed back on gather.
  Matmuls run in float32r (full PE rate, ~tf32 accuracy).

  v2 over the original baseline (77.0us -> ~52us marginal):
  - All K<=8 one-hot/broadcast matmuls (gate-row broadcast, b2 init,
    exp-sum, 1/sum broadcast) are packed 3-per-pass into disjoint 32-row
    strips of the PE array via tile_position so they overlap instead of
    each paying a full N=512 streaming pass. The gate logits are computed
    replicated 4x across partition strips (Wg2 replicated host-side) so
    every strip has the gate rows it needs.
  - The softmax normalization is applied once per token tile to the
    replicated exp rows (one DVE multiply against the PE-broadcast 1/sum),
    so expert outputs accumulate already-normalized in PSUM and the final
    evacuation is a ScalarE copy instead of a VectorE multiply - removing
    the end-of-tile DVE dependency that stalled the next tile's PSUM bank
    reuse.
  - exp-sum matmuls interleave into the gate phase; 1/sum broadcasts are
    software-pipelined one tile ahead so no PE stall waits on them.
  - All small constants (gate weights, one-hot masks, b1, b2 blocks) are
    fused into one [128, 1298] host-built tensor loaded with a single DMA.
  - PSUM: quad pool (3 banks) for gate/broadcast passes + 2 output-accum
    banks + 3 hidden-layer banks = 8. The 3 hidden banks (vs 2) give the
    ScalarE relu chain slack so the PE never waits on a phid bank.

  Measured pitfalls kept out of this design: putting the PSUM->SBUF output
  evacuation (or a fused per-expert gate multiply) on the VectorE queue
  regressed ~10-40us/iter - the in-order DVE queue head-of-line blocks the
  next tile's short-dependency multiplies behind a long tile-tail
  dependency. Both output copies stay on ScalarE.
"""
import numpy as np

import bass_rust
import concourse.bass as bass
import concourse.mybir as mybir
import concourse.tile as tile
from concourse.bass_utils import run_bass_kernel_spmd

F32 = mybir.dt.float32
F32R = mybir.dt.float32r
AF = mybir.ActivationFunctionType

N, D, E, H = 16384, 256, 8, 128
NCORES = 8
TPC = N // NCORES          # tokens per core
T = 512                    # token tile (max fp32 moving free dim)
NT = TPC // T              # token tiles per core
KC = D // 128              # 128-row chunks of the model dim

# fused gate/const tensor column layout ([128, GC_W] fp32)
GC_WG1 = 0                 # 256 cols: Wg1 as [p, kc*H + h]
GC_WG2 = 256               # 128 cols: Wg2 replicated into strips 32s+(0..7)
GC_BG1 = 384               # 1 col: bg1
GC_BG2 = 385               # 1 col: bg2 replicated into strips
GC_OH1 = 386               # 3x128 cols: one-hot rows; pass p strip j ->
                           # expert 3p+j (pass 2 only strips 0/1)
GC_OND = 770               # 128 cols: sum-selector; strip s rows have ones
                           # in column 32s only, so the exp-sum matmul's
                           # output lands at partition 32s with col group 0
GC_ON1 = 898               # 128 cols: ones rows at partitions {0,32,64,96}
GC_B2 = 1026               # 256 cols: b2 128-col blocks at strips 0 (mc=0), 1 (mc=1)
GC_B1 = 1282               # 16 cols: b1 as [p, e*KC + kc]
GC_W = 1298

# broadcast pass structure: expert e -> (pass e//3, strip e%3) except
# experts 6,7 -> (pass 2, strips 0,1)
def _pass_strip(e):
    return (e // 3, e % 3) if e < 6 else (2, e - 6)

_CTR = [0]


def _split_multi_waits(nc, max_waits=1):
    """This container's walrus rejects >1 sync-wait per instruction; hoist
    extras onto fresh same-engine NoOps placed just before the waiter."""
    for fn in nc.m.functions:
        for bb in fn.blocks:
            out = []
            for inst in bb.instructions:
                si = inst.sync_info
                waits = list(si.on_wait) if si is not None and si.on_wait else []
                if len(waits) > max_waits:
                    for w in waits[:-max_waits]:
                        _CTR[0] += 1
                        nop = bass_rust.InstNoOp(
                            name=f"I-waitfix-{_CTR[0]}", ins=[], outs=[])
                        nop.engine = inst.engine
                        nop.sync_info = mybir.SyncInfo(on_wait=[w], on_update=[])
                        nc.register_instruction(nop)
                        out.append(nop)
                    si.on_wait = waits[-max_waits:]
                out.append(inst)
            bb.instructions = out


def build_nc(repeat: int = 1):
    nc = bass.Bass("TRN2", target_bir_lowering=False, debug=False,
                   num_devices=NCORES)

    xT_d = nc.dram_tensor("xT", [D, TPC], F32, kind="ExternalInput")
    gc_d = nc.dram_tensor("gc", [128, GC_W], F32, kind="ExternalInput")
    W1_d = nc.dram_tensor("W1", [E, D, D], F32, kind="ExternalInput")
    W2_d = nc.dram_tensor("W2", [E, D, D], F32, kind="ExternalInput")
    yT_d = nc.dram_tensor("yT", [D, TPC], F32, kind="ExternalOutput")

    with tile.TileContext(nc) as tc:
        with (
            nc.allow_low_precision(reason="float32r matmul operands"),
            tc.tile_pool(name="wpool", bufs=1) as wp,
            tc.tile_pool(name="work", bufs=3) as sb,
            tc.tile_pool(name="gbuf", bufs=NT + 1) as gb,
            tc.tile_pool(name="hbuf", bufs=4) as hb,
            tc.tile_pool(name="obuf", bufs=4) as ob,
            tc.tile_pool(name="xpool", bufs=2) as xp,
            tc.tile_pool(name="quad", bufs=3, space="PSUM") as quad,
            tc.tile_pool(name="phid", bufs=3, space="PSUM") as phid,
            tc.tile_pool(name="pout", bufs=2, space="PSUM") as pout,
        ):
            gcx = wp.tile([128, GC_W], F32R, tag="gc")
            nc.sync.dma_start(gcx[:, :], gc_d[:, :].bitcast(F32R))

            def wg1_ap(kc):
                return gcx[:, GC_WG1 + kc * H:GC_WG1 + (kc + 1) * H]
            wg2r = gcx[:, GC_WG2:GC_WG2 + 128]
            bg1 = gcx[:, GC_BG1:GC_BG1 + 1].bitcast(F32)
            bg2r = gcx[:, GC_BG2:GC_BG2 + 1].bitcast(F32)

            def oh_ap(e):
                p, j = _pass_strip(e)
                base = GC_OH1 + 128 * p
                return gcx[32 * j:32 * j + 8, base:base + 128]

            def ond_ap(ti):
                return gcx[32 * ti:32 * ti + 8, GC_OND:GC_OND + 32 * ti + 1]

            def on1_ap(ti):
                return gcx[32 * ti:32 * ti + 1, GC_ON1:GC_ON1 + 128]

            def b2blk(mc):
                return gcx[32 * mc:32 * mc + 8,
                           GC_B2 + 128 * mc:GC_B2 + 128 * (mc + 1)]

            def b1bias(e, mc):
                c = GC_B1 + e * KC + mc
                return gcx[:, c:c + 1].bitcast(F32)

            w1 = wp.tile([128, E, KC, D], F32R, tag="w1")
            w2 = wp.tile([128, E, KC, D], F32R, tag="w2")

            def load_expert_weights(first_only=False, skip_first=False):
                w1src = W1_d.ap().rearrange(
                    "e (kc p) d -> p e kc d", p=128).bitcast(F32R)
                w2src = W2_d.ap().rearrange(
                    "e (kc p) d -> p e kc d", p=128).bitcast(F32R)
                for e in range(1 if skip_first else 0,
                               1 if first_only else E):
                    nc.sync.dma_start(w1[:, e, :, :], w1src[:, e, :, :])
                    nc.sync.dma_start(w2[:, e, :, :], w2src[:, e, :, :])

            def gate(xt, ti, rep, invs):
                """Gate logits/exp for tile ti, gate rows replicated x4
                across strips; exp-sum matmul on strip ti + reciprocal."""
                tok = slice(ti * T, (ti + 1) * T)
                pg1 = quad.tile([128, T], F32, tag="q", name=f"pg1_{rep}_{ti}")
                for kc in range(KC):
                    nc.tensor.matmul(pg1[:, :], wg1_ap(kc), xt[:, kc, tok],
                                     start=(kc == 0), stop=(kc == KC - 1))
                rh = sb.tile([H, T], F32R, tag="rh", name=f"rh_{rep}_{ti}")
                nc.scalar.activation(rh[:, :], pg1[:, :], AF.Relu, bias=bg1)
                pg2 = quad.tile([128, T], F32, tag="q", name=f"pg2_{rep}_{ti}")
                nc.tensor.matmul(pg2[:, :], wg2r, rh[:, :],
                                 start=True, stop=True)
                expl = gb.tile([128, T], F32R, tag="expl",
                               name=f"expl_{rep}_{ti}")
                nc.scalar.activation(expl[:, :], pg2[:, :], AF.Exp, bias=bg2r)
                qs = quad.tile([128, T], F32, tag="q", name=f"qs_{rep}_{ti}")
                nc.tensor.matmul(qs[0:32 * ti + 1, :], ond_ap(ti),
                                 expl[32 * ti:32 * ti + 8, :],
                                 start=True, stop=True,
                                 tile_position=(32 * ti, 0))
                nc.vector.reciprocal(invs[32 * ti:32 * ti + 1, :],
                                     qs[32 * ti:32 * ti + 1, :])
                return expl

            def normalize(ti, rep, invs, expl):
                """Broadcast 1/sum to 128 partitions (K=1 matmul on strip ti)
                and scale the replicated exp rows in place: expl becomes the
                normalized gate, so downstream accumulation needs no final
                renormalization."""
                pv = quad.tile([128, T], F32, tag="q", name=f"pv_{rep}_{ti}")
                nc.tensor.matmul(pv[:, :], on1_ap(ti),
                                 invs[32 * ti:32 * ti + 1, :],
                                 start=True, stop=True,
                                 tile_position=(32 * ti, 0))
                nc.vector.tensor_mul(expl[:, :], expl[:, :], pv[:, :])

            def experts_compute(xt, ti, rep, expl, post_e1=None):
                tok = slice(ti * T, (ti + 1) * T)

                def bcast_pass(base):
                    out = []
                    for e in range(base, min(base + 3, E)):
                        _, j = _pass_strip(e)
                        pt = quad.tile([128, T], F32, tag="q",
                                       name=f"pgb_{rep}_{ti}_{e}")
                        nc.tensor.matmul(pt[:, :], oh_ap(e),
                                         expl[32 * j:32 * j + 8, :],
                                         start=True, stop=True,
                                         tile_position=(32 * j, 0))
                        out.append(pt)
                    return out

                pgb = bcast_pass(0)
                py = None
                for e in range(E):
                    if e in (3, 6):
                        pgb = bcast_pass(e)
                    if e == 2 and post_e1 is not None:
                        post_e1()
                    pt = pgb[e % 3 if e < 6 else e - 6]
                    hs = hb.tile([128, KC, T], F32R, tag="hs",
                                 name=f"hs_{rep}_{ti}_{e}")
                    for mc in range(KC):
                        ph = phid.tile([128, T], F32, tag="ph",
                                       name=f"ph_{rep}_{ti}_{e}_{mc}")
                        for kc in range(KC):
                            nc.tensor.matmul(
                                ph[:, :], w1[:, e, kc, mc * 128:(mc + 1) * 128],
                                xt[:, kc, tok],
                                start=(kc == 0), stop=(kc == KC - 1))
                        nc.scalar.activation(hs[:, mc, :], ph[:, :], AF.Relu,
                                             bias=b1bias(e, mc))
                        nc.vector.tensor_mul(hs[:, mc, :], hs[:, mc, :],
                                             pt[:, :])
                    if e == 0:
                        # b2 init after e0's first-layer matmuls: gives the
                        # previous tile's output copies time to free the
                        # banks before the accumulation group opens.
                        py = [pout.tile([128, T], F32, tag="py",
                                        name=f"py{mc}_{rep}_{ti}")
                              for mc in range(KC)]
                        for mc in range(KC):
                            nc.tensor.matmul(py[mc][:, :], b2blk(mc),
                                             expl[32 * mc:32 * mc + 8, :],
                                             start=True, stop=False,
                                             tile_position=(32 * mc, 0))
                    for mc in range(KC):
                        for kc in range(KC):
                            nc.tensor.matmul(
                                py[mc][:, :],
                                w2[:, e, kc, mc * 128:(mc + 1) * 128],
                                hs[:, kc, :],
                                start=False,
                                stop=(e == E - 1 and kc == KC - 1))
                return py

            def finalize(ti, rep, py):
                tok = slice(ti * T, (ti + 1) * T)
                for mc in range(KC):
                    ot = ob.tile([128, T], F32, tag="ot",
                                 name=f"ot_{rep}_{ti}_{mc}")
                    nc.scalar.activation(ot[:, :], py[mc][:, :], AF.Copy)
                    nc.gpsimd.dma_start(yT_d[mc * 128:(mc + 1) * 128, tok],
                                        ot[:, :])

            for rep in range(repeat):
                xt = xp.tile([128, KC, TPC], F32R, tag="xt", name=f"xt{rep}")
                xsrc = xT_d.ap().rearrange(
                    "(kc p) t -> p kc t", p=128).bitcast(F32R)
                nc.sync.dma_start(xt[:, :, 0:T], xsrc[:, :, 0:T])
                if rep == 0:
                    load_expert_weights(first_only=True)
                for ti in range(1, NT):
                    tok = slice(ti * T, (ti + 1) * T)
                    nc.sync.dma_start(xt[:, :, tok], xsrc[:, :, tok])
                invs = sb.tile([128, T], F32R, tag="invs", name=f"invs_{rep}")
                expls = []
                for ti in range(NT):
                    expls.append(gate(xt, ti, rep, invs))
                    # normalize lags one tile behind the gate so the
                    # reciprocal is ready and the PE never waits on it
                    if ti >= 1:
                        normalize(ti - 1, rep, invs, expls[ti - 1])
                if rep == 0:
                    load_expert_weights(skip_first=True)
                for ti in range(NT):
                    # the last tile's normalize is deferred into tile 0's
                    # expert phase so its reciprocal has time to finish and
                    # its PSUM bank never gates the broadcast-pass ring
                    post = ((lambda: normalize(NT - 1, rep, invs,
                                               expls[NT - 1]))
                            if ti == 0 else None)
                    py = experts_compute(xt, ti, rep, expls[ti],
                                         post_e1=post)
                    finalize(ti, rep, py)

    _split_multi_waits(nc)
    return nc


_NC_CACHE = None


def _get_nc():
    global _NC_CACHE
    if _NC_CACHE is None:
        _NC_CACHE = build_nc()
    return _NC_CACHE


def make_in_maps(x, Wg1, bg1, Wg2, bg2, W1, b1, W2, b2):
    x = np.ascontiguousarray(np.asarray(x, dtype=np.float32))
    xT = np.ascontiguousarray(x.T)           # [D, N]
    Wg1 = np.asarray(Wg1, np.float32)
    bg1 = np.asarray(bg1, np.float32)
    Wg2 = np.asarray(Wg2, np.float32)
    bg2 = np.asarray(bg2, np.float32)
    b1 = np.asarray(b1, np.float32)
    b2 = np.asarray(b2, np.float32)

    gc = np.zeros((128, GC_W), np.float32)
    # Wg1 [D, H] -> [p, kc*H + h]
    gc[:, GC_WG1:GC_WG1 + KC * H] = (
        Wg1.reshape(KC, 128, H).transpose(1, 0, 2).reshape(128, KC * H))
    # Wg2 replicated: wg2r[h, 32s+k] = Wg2[h, k]; bg2 likewise per strip
    for s in range(4):
        gc[:, GC_WG2 + 32 * s:GC_WG2 + 32 * s + 8] = Wg2
        gc[32 * s:32 * s + 8, GC_BG2] = bg2
    gc[:, GC_BG1] = bg1
    # one-hot strips: expert e lives in pass e//3 at strip e%3 (6,7: pass 2
    # strips 0/1); within its [8,128] strip block, row e is ones
    for e in range(E):
        p, j = _pass_strip(e)
        gc[32 * j + e, GC_OH1 + 128 * p:GC_OH1 + 128 * (p + 1)] = 1.0
    for j in range(4):
        gc[32 * j:32 * j + 8, GC_OND + 32 * j] = 1.0
        gc[32 * j, GC_ON1:GC_ON1 + 128] = 1.0
    # b2 blocks: strip mc holds b2[:, mc*128:(mc+1)*128]
    for mc in range(KC):
        gc[32 * mc:32 * mc + 8,
           GC_B2 + 128 * mc:GC_B2 + 128 * (mc + 1)] = b2[:, mc * 128:(mc + 1) * 128]
    # b1 as [p, e*KC + kc]
    gc[:, GC_B1:GC_B1 + E * KC] = (
        b1.reshape(E, KC, 128).transpose(2, 0, 1).reshape(128, E * KC))

    shared = {
        "gc": np.ascontiguousarray(gc),
        "W1": np.ascontiguousarray(np.asarray(W1, np.float32)),
        "W2": np.ascontiguousarray(np.asarray(W2, np.float32)),
    }
    return [
        {"xT": np.ascontiguousarray(xT[:, c * TPC:(c + 1) * TPC]), **shared}
        for c in range(NCORES)
    ]


def gather_output(results):
    out = np.empty((N, D), np.float32)
    for c in range(NCORES):
        out[c * TPC:(c + 1) * TPC, :] = results[c]["yT"].T
    return out


def kernel(x, Wg1, bg1, Wg2, bg2, W1, b1, W2, b2):
    nc = _get_nc()
    in_maps = make_in_maps(x, Wg1, bg1, Wg2, bg2, W1, b1, W2, b2)
    r = run_bass_kernel_spmd(nc, in_maps, list(range(NCORES)))
    return gather_output(r.results)
